# revision 86
# baseline (speedup 1.0000x reference)
"""Trainium2 Bass kernel for a dense pre-LN transformer block.

B=4, T=1024, C=1024, H=16 heads (head_size 64).

Distribution over the 8 NeuronCores (two SPMD launches, host-side
reshuffle between them):

  Launch A (attention + partial proj): core c handles batch c//2 and
  head-group c%2 (8 heads). Each core LNs only its own batch, computes
  its heads' QKV (fp8 DoubleRow), scores (bf16), softmax (exp on ACT,
  fp8 out), AV (fp8), and the partial output projection for its 512
  Wo rows; the half-partials stream to DRAM.
  NOTE the reference computes scores as k @ q^T (roles of q/k swapped
  vs standard attention) — handled by using q rows as score partitions.

  Host: proj_full = sum of the 4 partials per batch; x2 = x + proj + bo.

  Launch B (FFN, row-parallel): core c runs LN2 + W1/PReLU/W2 (bf16;
  fp8 W1/W2 measured over the 2e-2 error gate) + residual on rows
  [512c, 512(c+1)).

LayerNorm applies on the vector engine. PReLU(x) = x + (1-a)*Relu(-x),
split ACT/DVE (one PSUM read each). fp8 weights are pre-scaled by
QS=64 to clear e4m3 subnormals; the scale cancels via the exp scale
(q,k) and the scaled ones-column in the softmax denominator (v).
"""

import os
from contextlib import ExitStack

import numpy as np

import concourse.bass as bass
import concourse.tile as tile
from concourse import bacc, mybir
from concourse.bass_utils import run_bass_kernel_spmd
from concourse.masks import make_identity

F32 = mybir.dt.float32
F32R = mybir.dt.float32r
BF16 = mybir.dt.bfloat16
F8 = mybir.dt.float8e4
DR = mybir.MatmulPerfMode.DoubleRow
AF = mybir.ActivationFunctionType
ALU = mybir.AluOpType

# fp8(e4m3) weight pre-scale: lifts the 0.02-rms weights out of the
# subnormal range (min normal 2^-6). For QKV the q/k factors cancel in
# softmax via the exp scale; the v factor cancels against the scaled
# ones-column in the softmax denominator.
QS = 64.0

B, T, C, H, HS = 4, 1024, 1024, 16, 64
NCORES = 8
EPS = 1e-5
SCALE = float(C) ** -0.5  # 1/32, folded into the softmax exp
NEG = -1e30

NTB = T // 128   # 8 token blocks per batch
NCC = C // 128   # 8 channel chunks
HPC = 8          # heads per core


def _bf(x):
    import ml_dtypes
    return np.ascontiguousarray(np.asarray(x, np.float32).astype(
        ml_dtypes.bfloat16))


def _f8(x, scale=1.0):
    import ml_dtypes
    return np.ascontiguousarray(
        (np.asarray(x, np.float32) * scale).astype(ml_dtypes.float8_e4m3))


# --------------------------------------------------------------------------
# kernel A: attention, one batch + 8 heads per core
# --------------------------------------------------------------------------

def _attn_body(ctx, tc, x, wq, wk, wv, lnw, lnb, woh, ppart):
    """Per-core: LN1 on its batch, QKV/scores/AV for its 8 heads, then
    the partial output projection for its head-group channels.

    Scores are built transposed (s on partitions, t on free dim) so the
    softmax denominator comes from an appended ones-column in v; AV
    output lands as [t, d] tiles which are normalized four heads at a
    time (strided reciprocal + broadcast multiply), then PE-transposed
    into catT (kept in SBUF). Each half's catT rows immediately feed
    partial-proj matmuls against the core's 512 Wo rows; the two
    half-partials stream to DRAM and the host sums the four partials
    per batch (launch B then skips proj entirely).
    """
    nc = tc.nc
    general_ln = lnw is not None

    const = ctx.enter_context(tc.tile_pool(name="const", bufs=1))
    scratch = const.tile([128, 128], F32)
    make_identity(nc, scratch)
    ident = const.tile([128, 128], BF16)
    nc.vector.tensor_copy(out=ident, in_=scratch)
    # transposed causal 0/1 mask for diagonal blocks: keep s<=t
    trilT = const.tile([128, 128], BF16)
    nc.gpsimd.memset(trilT, 1.0)
    nc.gpsimd.affine_select(
        out=trilT, in_=trilT, compare_op=ALU.is_ge, fill=0.0, base=0,
        pattern=[[1, 128]], channel_multiplier=-1)
    trilT8 = const.tile([128, 128], F8)
    nc.vector.tensor_copy(out=trilT8, in_=trilT)
    eps_t = const.tile([128, 1], F32)
    nc.vector.memset(eps_t, EPS)

    wq_sb = const.tile([128, 4, 2, 512], F8, tag="wq")
    wk_sb = const.tile([128, 4, 2, 512], F8, tag="wk")
    wv_sb = const.tile([128, 4, 2, 512], F8, tag="wv")
    # weights ride the (otherwise idle) sync queue, delayed behind the
    # first x tiles; keeps the ACT engine free for the LN sqrt chain
    with tc.tile_wait_until(0.003):
        nc.sync.dma_start(
            out=wq_sb, in_=wq.rearrange("(a j p) d -> p a j d", j=2, p=128))
        nc.sync.dma_start(
            out=wk_sb, in_=wk.rearrange("(a j p) d -> p a j d", j=2, p=128))
        nc.sync.dma_start(
            out=wv_sb, in_=wv.rearrange("(a j p) d -> p a j d", j=2, p=128))
    woh_sb = const.tile([128, 4, C], BF16, tag="woh")
    with tc.tile_wait_until(0.006):
        nc.sync.dma_start(
            out=woh_sb, in_=woh.rearrange("(dc p) n -> p dc n", p=128))
    if general_ln:
        lnw_bc = const.tile([128, C], F32, tag="lnw")
        lnb_bc = const.tile([128, C], F32, tag="lnb")
        nc.sync.dma_start(
            out=lnw_bc,
            in_=bass.AP(tensor=lnw.tensor, offset=lnw.offset,
                        ap=[[0, 128]] + list(lnw.ap)))
        nc.sync.dma_start(
            out=lnb_bc,
            in_=bass.AP(tensor=lnb.tensor, offset=lnb.offset,
                        ap=[[0, 128]] + list(lnb.ap)))

    xp = ctx.enter_context(tc.tile_pool(name="xp", bufs=3))
    hp = ctx.enter_context(tc.tile_pool(name="hp", bufs=3))
    hTp = ctx.enter_context(tc.tile_pool(name="hTp", bufs=1))
    stat = ctx.enter_context(tc.tile_pool(name="stat", bufs=3))
    qkp = ctx.enter_context(tc.tile_pool(name="qkp", bufs=3))
    vp = ctx.enter_context(tc.tile_pool(name="vp", bufs=3))
    epl = ctx.enter_context(tc.tile_pool(name="epl", bufs=5))
    ctkp = ctx.enter_context(tc.tile_pool(name="ctkp", bufs=NTB))
    recp = ctx.enter_context(tc.tile_pool(name="recp", bufs=4))
    catp = ctx.enter_context(tc.tile_pool(name="catp", bufs=1))
    ppsb = ctx.enter_context(tc.tile_pool(name="ppsb", bufs=2))

    # 4 PSM banks let QKV keep one accumulation group per bank while the
    # stationary fp8 weight tile is reused across all four token
    # quarters (one LDWEIGHTS per 4 matmuls). Proj shares PAV slots.
    PSM = ctx.enter_context(tc.tile_pool(name="psm", bufs=4, space="PSUM"))
    PSS = ctx.enter_context(tc.tile_pool(name="pss", bufs=2, space="PSUM"))
    PAV = ctx.enter_context(tc.tile_pool(name="pav", bufs=2, space="PSUM"))

    # ---- LN1, fully pipelined per token tile ----
    hT = hTp.tile([128, NCC, T], F8, tag="hT")
    for i in range(NTB):
        xt = xp.tile([128, C], F32, tag="x", name=f"x_{i}")
        # alternate HW DMA queues so early tiles land without queueing
        # behind the full 4MB of x (gpsimd's software queue is slow and
        # stalls the engine that builds ident/trilT)
        (nc.sync if i % 2 == 0 else nc.scalar).dma_start(
            out=xt, in_=x[i * 128:(i + 1) * 128, :])
        st = stat.tile([128, 2, 6], F32, tag="bn", name=f"bn_{i}")
        for k in range(2):
            nc.vector.bn_stats(out=st[:, k, :], in_=xt[:, k * 512:(k + 1) * 512])
        mv = stat.tile([128, 2], F32, tag="mv", name=f"mv_{i}")
        nc.vector.bn_aggr(out=mv, in_=st)
        std = stat.tile([128, 1], F32, tag="std", name=f"std_{i}")
        nc.scalar.activation(out=std, in_=mv[:, 1:2], func=AF.Sqrt, bias=eps_t)
        rstd = stat.tile([128, 1], F32, tag="rstd", name=f"rstd_{i}")
        nc.vector.reciprocal(out=rstd, in_=std)
        ht = hp.tile([128, C], BF16, tag="h", name=f"h_{i}")
        if general_ln:
            hf = hp.tile([128, C], F32, tag="hf", name=f"hf_{i}")
            nc.vector.tensor_scalar(
                out=hf, in0=xt, scalar1=mv[:, 0:1],
                scalar2=rstd, op0=ALU.subtract, op1=ALU.mult)
            nc.vector.tensor_mul(out=hf, in0=hf, in1=lnw_bc)
            nc.vector.tensor_add(out=ht, in0=hf, in1=lnb_bc)
        else:
            nc.vector.tensor_scalar(
                out=ht, in0=xt, scalar1=mv[:, 0:1],
                scalar2=rstd, op0=ALU.subtract, op1=ALU.mult)
        # transpose this tile into hT right away: 8 cc blocks -> one bank
        pt = PSM.tile([128, 1024], BF16, tag="mm", name=f"pt_{i}")
        for cc in range(NCC):
            nc.tensor.transpose(
                pt[:, cc * 128:(cc + 1) * 128],
                ht[:, cc * 128:(cc + 1) * 128], ident)
        nc.vector.tensor_copy(
            out=hT[:, :, i * 128:(i + 1) * 128],
            in_=pt.rearrange("p (cc q) -> p cc q", cc=NCC))

    cat_toks = [ctkp.tile([128, 512], BF16, tag="ctk", name=f"ctk_{i}")
                for i in range(NTB)]
    catT = catp.tile([128, 4, T], BF16, tag="catT")

    # ---- per half (= 2 head pairs): qkv, scores, AV, norm ----
    for half in range(2):
        e_all = {}   # (h4, sc) -> e tile
        v2s = []
        for pp_ in range(2):
            p = half * 2 + pp_
            psl = slice(p * 128, (p + 1) * 128)
            qT2 = qkp.tile([128, T], BF16, tag="qT", name=f"qT_{p}")
            kT2 = qkp.tile([128, T], BF16, tag="kT", name=f"kT_{p}")

            def qkv_mm(w_sb, dst, nm, on_act):
                # fp8 DoubleRow, a4-outer: each stationary weight tile
                # is loaded once and fed all four 256-token quarters
                # (one open accumulation group per PSUM bank - safe)
                pqs = [PSM.tile([128, 256], F32, tag="mm",
                                name=f"p{nm}_{p}_{tq}") for tq in range(4)]
                for a4 in range(4):
                    for tq in range(4):
                        nc.tensor.matmul(
                            pqs[tq], w_sb[:, a4, :, psl],
                            hT[:, 2 * a4:2 * a4 + 2,
                               tq * 256:(tq + 1) * 256],
                            start=(a4 == 0), stop=(a4 == 3), perf_mode=DR)
                for tq in range(4):
                    osl = slice(tq * 256, (tq + 1) * 256)
                    if on_act:
                        nc.scalar.copy(out=dst[:, osl], in_=pqs[tq])
                    else:
                        nc.vector.tensor_copy(out=dst[:, osl], in_=pqs[tq])

            qkv_mm(wq_sb, qT2, "q", True)
            qkv_mm(wk_sb, kT2, "k", True)
            # v2: [t_part, sc, 132]: per head 66 cols (64 v + ones + pad).
            # Compute vT with wide matmuls (stationary wv reused), then
            # PE-transpose blocks back to [t, d] for the AV rhs.
            vT2 = qkp.tile([128, T], BF16, tag="vT", name=f"vT_{p}")
            qkv_mm(wv_sb, vT2, "v", False)
            v2 = vp.tile([128, NTB, 132], F8, tag="v2", name=f"v2_{p}")
            nc.vector.memset(v2[:, :, 64:66], 0.0)
            nc.vector.memset(v2[:, :, 130:132], 0.0)
            # ones-column carries the same QS scale as v so the
            # normalization by the denominator cancels it exactly
            nc.vector.memset(v2[:, :, 64:65], QS)
            nc.vector.memset(v2[:, :, 130:131], QS)
            for g in range(2):
                ptv = PSM.tile([128, 512], BF16, tag="mm", name=f"ptv_{p}_{g}")
                for j in range(4):
                    i = g * 4 + j
                    nc.tensor.transpose(
                        ptv[:, j * 128:(j + 1) * 128],
                        vT2[:, i * 128:(i + 1) * 128], ident)
                pv4 = ptv.rearrange("p (j two d) -> p j two d", j=4, two=2)
                nc.vector.tensor_copy(
                    out=v2[:, g * 4:(g + 1) * 4, 0:64], in_=pv4[:, :, 0, :])
                nc.vector.tensor_copy(
                    out=v2[:, g * 4:(g + 1) * 4, 66:130], in_=pv4[:, :, 1, :])
            v2s.append(v2)

            # scores + exp for the two heads of this pair. e lands fp8 in
            # E[h4] laid out [scpair, j, i, t] so AV can run DoubleRow
            # over s-block pairs.
            for hh in range(2):
                h4 = pp_ * 2 + hh
                hsl = slice(hh * 64, (hh + 1) * 64)
                Eh = epl.tile([128, 4, 2, NTB, 128], F8, tag="E",
                              name=f"E_{half}_{h4}")
                e_all[h4] = Eh
                for sc in range(NTB):
                    W = (NTB - sc) * 128
                    g, j = sc // 2, sc % 2
                    n0 = 0
                    while n0 < W:
                        n1 = min(n0 + 512, W)
                        pss = PSS.tile([128, n1 - n0], F32, tag="score",
                                       name=f"pss_{p}_{hh}_{sc}_{n0}")
                        nc.tensor.matmul(
                            pss,
                            qT2[hsl, sc * 128:(sc + 1) * 128],
                            kT2[hsl, sc * 128 + n0:sc * 128 + n1],
                            start=True, stop=True)
                        nc.scalar.activation(
                            out=Eh[:, g, j, sc + n0 // 128:sc + n1 // 128,
                                   :],
                            in_=pss, func=AF.Exp,
                            scale=SCALE / (QS * QS))
                        n0 = n1
                    # causal mask on the diagonal block: cheap fp8 SBUF
                    # multiply after exp (exp of unmasked scores is benign)
                    nc.vector.tensor_mul(out=Eh[:, g, j, sc, :],
                                         in0=Eh[:, g, j, sc, :], in1=trilT8)

        # AV: 4 heads batched into one [128, 264] psum per token block.
        # Plain fp8 matmuls: DoubleRow here loses — the paired stationary
        # doubles LDWEIGHTS, which dominates these 66-row matmuls.
        for i in range(NTB):
            po4 = PAV.tile([128, 4, 66], F32, tag="po4", name=f"po4_{half}_{i}")
            for h4 in range(4):
                hh = h4 % 2
                v2 = v2s[h4 // 2]
                Eh = e_all[h4]
                for sc in range(i + 1):
                    nc.tensor.matmul(
                        po4[:, h4, :],
                        Eh[:, sc // 2, sc % 2, i, :],
                        v2[:, sc, hh * 66:(hh + 1) * 66],
                        start=(sc == 0), stop=(sc == i))
            rec4 = recp.tile([128, 4], F32, tag="rec", name=f"rec_{half}_{i}")
            nc.vector.reciprocal(out=rec4, in_=po4[:, :, 64:65].rearrange(
                "p a b -> p (a b)"))
            rec_bc = bass.AP(tensor=rec4.tensor, offset=rec4.offset,
                             ap=[list(rec4.ap[0]), [1, 4], [0, 64]])
            nc.vector.tensor_tensor(
                out=cat_toks[i][:, half * 256:(half + 1) * 256].rearrange(
                    "p (a b) -> p a b", a=4),
                in0=po4[:, :, 0:64], in1=rec_bc, op=ALU.mult)

        # transpose this half's cat columns -> catT rows (SBUF only)
        for ccc in (2 * half, 2 * half + 1):
            for g in range(2):
                pt = PSM.tile([128, 512], BF16, tag="mm",
                              name=f"ptc_{ccc}_{g}")
                for j in range(4):
                    i = g * 4 + j
                    nc.tensor.transpose(
                        pt[:, j * 128:(j + 1) * 128],
                        cat_toks[i][:, ccc * 128:(ccc + 1) * 128], ident)
                nc.vector.tensor_copy(
                    out=catT[:, ccc, g * 512:(g + 1) * 512], in_=pt)
        # partial proj for this half's two d-chunks: overlaps the other
        # half's (ACT-bound) score/AV work, and launch B drops proj.
        # All 16 pieces gather in one SBUF tile -> a single 2MB DMA.
        pph = ppsb.tile([128, 2, NTB, 512], BF16, tag="pph",
                        name=f"pph_{half}")
        ppart_r = ppart[half].rearrange("(i p) (co n) -> p co i n",
                                        p=128, co=2)
        for co in range(2):
            for i in range(NTB):
                ppp = PSS.tile([128, 512], F32, tag="score",
                               name=f"pp_{half}_{i}_{co}")
                for k, dc in enumerate((2 * half, 2 * half + 1)):
                    nc.tensor.matmul(
                        ppp, catT[:, dc, i * 128:(i + 1) * 128],
                        woh_sb[:, dc, co * 512:(co + 1) * 512],
                        start=(k == 0), stop=(k == 1))
                nc.vector.tensor_copy(out=pph[:, co, i, :], in_=ppp)
                if i == 3:  # first half of the rows streams out early
                    nc.scalar.dma_start(out=ppart_r[:, co, 0:4],
                                        in_=pph[:, co, 0:4])
            nc.scalar.dma_start(out=ppart_r[:, co, 4:8],
                                in_=pph[:, co, 4:8])


def _build_attn(general_ln: bool):
    nc = bacc.Bacc("TRN2", target_bir_lowering=False, debug=False)
    x = nc.dram_tensor("x", [T, C], F32, kind="ExternalInput").ap()
    wq = nc.dram_tensor("wq", [C, 512], F8, kind="ExternalInput").ap()
    wk = nc.dram_tensor("wk", [C, 512], F8, kind="ExternalInput").ap()
    wv = nc.dram_tensor("wv", [C, 512], F8, kind="ExternalInput").ap()
    lnw = lnb = None
    if general_ln:
        lnw = nc.dram_tensor("lnw", [C], F32, kind="ExternalInput").ap()
        lnb = nc.dram_tensor("lnb", [C], F32, kind="ExternalInput").ap()
    woh = nc.dram_tensor("woh", [512, C], BF16, kind="ExternalInput").ap()
    ppart = nc.dram_tensor("ppart", [2, T, C], BF16,
                           kind="ExternalOutput").ap()
    with tile.TileContext(nc) as tc:
        with ExitStack() as ctx:
            _attn_body(ctx, tc, x, wq, wk, wv, lnw, lnb, woh, ppart)
    nc.compile()
    return nc


# --------------------------------------------------------------------------
# kernel B: FFN, 512 rows per core
# --------------------------------------------------------------------------

RPC = (B * T) // NCORES  # 512 rows per core
NRB = RPC // 128         # 4 row blocks
NHID = 4 * C // 128      # 32 hidden chunks


def _ffn_body(ctx, tc, x2in, w1, w2, b1, ln2w, ln2b, b2, alpha, out):
    """Per-core rows: LN2 + FFN on host-precomputed x2 = x + proj + bo.
    The projection itself happens in launch A."""
    nc = tc.nc
    general_ln = ln2w is not None

    const = ctx.enter_context(tc.tile_pool(name="const", bufs=1))
    scratch = const.tile([128, 128], F32)
    make_identity(nc, scratch)
    ident = const.tile([128, 128], BF16)
    nc.vector.tensor_copy(out=ident, in_=scratch)
    eps_t = const.tile([128, 1], F32)
    nc.vector.memset(eps_t, EPS)
    b1_sb = None
    if b1 is not None:
        b1_sb = const.tile([128, NHID], F32, tag="b1")
        nc.sync.dma_start(out=b1_sb, in_=b1.rearrange("(h p) -> p h", p=128))

    def bcast(src, tag):
        t = const.tile([128, C], F32, tag=tag, name=tag)
        nc.sync.dma_start(
            out=t, in_=bass.AP(tensor=src.tensor, offset=src.offset,
                               ap=[[0, 128]] + list(src.ap)))
        return t

    lnw_bc = bcast(ln2w, "lnw") if general_ln else None
    lnb_bc = bcast(ln2b, "lnb") if general_ln else None
    b2_bc = bcast(b2, "b2") if b2 is not None else None

    x2p = ctx.enter_context(tc.tile_pool(name="x2p", bufs=NRB))
    hp = ctx.enter_context(tc.tile_pool(name="hp", bufs=5))
    h2Tp = ctx.enter_context(tc.tile_pool(name="h2Tp", bufs=1))
    stat = ctx.enter_context(tc.tile_pool(name="stat", bufs=2))
    w1p = ctx.enter_context(tc.tile_pool(name="w1p", bufs=2))
    w2p = ctx.enter_context(tc.tile_pool(name="w2p", bufs=2))
    w2bp = ctx.enter_context(tc.tile_pool(name="w2bp", bufs=NHID // 4))
    ftp = ctx.enter_context(tc.tile_pool(name="ftp", bufs=NHID))
    tmp = ctx.enter_context(tc.tile_pool(name="tmp", bufs=3))
    osb = ctx.enter_context(tc.tile_pool(name="osb", bufs=1))

    # x2 tiles alternate the two HW queues so LN2 starts immediately
    x2_tiles = []
    for r in range(NRB):
        x2t = x2p.tile([128, C], F32, tag="x2", name=f"x2_{r}")
        (nc.sync if r % 2 == 0 else nc.scalar).dma_start(
            out=x2t, in_=x2in[r * 128:(r + 1) * 128, :])
        x2_tiles.append(x2t)

    # ---- LN2 + transpose, pipelined per row tile ----
    h2T = h2Tp.tile([128, NCC, RPC], BF16, tag="h2T")
    with tc.tile_pool(name="pst", bufs=2, space="PSUM") as PST:
        for r in range(NRB):
            st = stat.tile([128, 2, 6], F32, tag="bn", name=f"bn_{r}")
            for k in range(2):
                nc.vector.bn_stats(out=st[:, k, :],
                                   in_=x2_tiles[r][:, k * 512:(k + 1) * 512])
            mv = stat.tile([128, 2], F32, tag="mv", name=f"mv_{r}")
            nc.vector.bn_aggr(out=mv, in_=st)
            std = stat.tile([128, 1], F32, tag="std", name=f"std_{r}")
            nc.scalar.activation(out=std, in_=mv[:, 1:2], func=AF.Sqrt,
                                 bias=eps_t)
            rstd = stat.tile([128, 1], F32, tag="rstd", name=f"rstd_{r}")
            nc.vector.reciprocal(out=rstd, in_=std)
            ht = hp.tile([128, C], BF16, tag="h", name=f"h_{r}")
            if general_ln:
                hf = hp.tile([128, C], F32, tag="hf", name=f"hf_{r}")
                nc.vector.tensor_scalar(
                    out=hf, in0=x2_tiles[r], scalar1=mv[:, 0:1],
                    scalar2=rstd, op0=ALU.subtract, op1=ALU.mult)
                nc.vector.tensor_mul(out=hf, in0=hf, in1=lnw_bc)
                nc.vector.tensor_add(out=ht, in0=hf, in1=lnb_bc)
            else:
                nc.vector.tensor_scalar(
                    out=ht, in0=x2_tiles[r], scalar1=mv[:, 0:1],
                    scalar2=rstd, op0=ALU.subtract, op1=ALU.mult)
            pt = PST.tile([128, 1024], BF16, tag="tr", name=f"pt_{r}")
            for cc in range(NCC):
                nc.tensor.transpose(pt[:, cc * 128:(cc + 1) * 128],
                                    ht[:, cc * 128:(cc + 1) * 128], ident)
            nc.vector.tensor_copy(
                out=h2T[:, :, r * 128:(r + 1) * 128],
                in_=pt.rearrange("p (cc q) -> p cc q", cc=NCC))

    # ---- W1 + PReLU + W2 (col-half 0), then W2 col-half 1 ----
    # bf16 throughout: fp8(e4m3) DoubleRow was measured at rel err
    # 1.9e-2 (W1 or W2 alone) to 2.7e-2 (both) vs the 2e-2 gate.
    NG = NHID // 4
    f_tiles = []
    w1gs = []
    with tc.tile_wait_until(0.002):
        for g in range(NG):
            w1g = w1p.tile([128, 4, NCC, 128], BF16, tag="w1", name=f"w1_{g}")
            nc.sync.dma_start(out=w1g, in_=w1[g])
            w1gs.append(w1g)
    w2gs0 = []
    with tc.tile_wait_until(0.010):
        for g in range(NG):
            w2g = w2p.tile([128, 4, 512], BF16, tag="w2a", name=f"w2a_{g}")
            nc.scalar.dma_start(
                out=w2g,
                in_=w2[g * 512:(g + 1) * 512, 0:512].rearrange(
                    "(hh p) n -> p hh n", p=128))
            w2gs0.append(w2g)
    w2g1s = []
    with tc.tile_wait_until(0.018):
        for g in range(NG):
            w2g1 = w2bp.tile([128, 4, 512], BF16, tag="w2b", name=f"w2b_{g}")
            nc.scalar.dma_start(
                out=w2g1,
                in_=w2[g * 512:(g + 1) * 512, 512:1024].rearrange(
                    "(hh p) n -> p hh n", p=128))
            w2g1s.append(w2g1)
    out_r = out.rearrange("(r p) c -> p r c", p=128)
    o_sb = osb.tile([128, NRB, C], F32, tag="o")
    # col-half 1 rows 0-1 accumulate inline with the W1 loop (8 PSUM
    # banks: 2 pf + 4 half0 + 2 half1a); only rows 2-3 trail the loop.
    with tc.tile_pool(name="psf", bufs=2, space="PSUM") as PSF, \
         tc.tile_pool(name="pso", bufs=4, space="PSUM") as PSO, \
         tc.tile_pool(name="pso1", bufs=2, space="PSUM") as PSO1:
        pouts0 = [PSO.tile([128, 512], F32, tag="out0", name=f"po0_{r}")
                  for r in range(NRB)]
        pouts1 = [PSO1.tile([128, 512], F32, tag="out1", name=f"po1_{r}")
                  for r in range(2)]
        for h in range(NHID):
            g, hh = h // 4, h % 4
            pf = PSF.tile([128, RPC], F32, tag="ft", name=f"pf_{h}")
            for cc in range(NCC):
                nc.tensor.matmul(pf, w1gs[g][:, hh, cc, :], h2T[:, cc, :],
                                 start=(cc == 0), stop=(cc == NCC - 1))
            ft = ftp.tile([128, RPC], BF16, tag="ft", name=f"ft_{h}")
            if b1_sb is not None:
                src = tmp.tile([128, RPC], F32, tag="pb", name=f"pb_{h}")
                nc.vector.tensor_scalar_add(out=src, in0=pf,
                                            scalar1=b1_sb[:, h:h + 1])
            else:
                src = pf
            # PReLU(x) = x + (1-a)*Relu(-x); Relu on ACT (one PSUM read),
            # add on DVE (one PSUM read) — BIR allows max one PSUM input.
            rneg = tmp.tile([128, RPC], BF16, tag="rneg", name=f"rn_{h}")
            nc.scalar.activation(out=rneg, in_=src, func=AF.Relu,
                                 scale=alpha - 1.0)
            nc.vector.tensor_add(out=ft, in0=src, in1=rneg)
            f_tiles.append(ft)
            for r in range(NRB):
                nc.tensor.matmul(pouts0[r], ft[:, r * 128:(r + 1) * 128],
                                 w2gs0[g][:, hh, :],
                                 start=(h == 0), stop=(h == NHID - 1))
            for r in range(2):
                nc.tensor.matmul(pouts1[r], ft[:, r * 128:(r + 1) * 128],
                                 w2g1s[g][:, hh, :],
                                 start=(h == 0), stop=(h == NHID - 1))
        for r in range(NRB):
            nc.vector.tensor_add(out=o_sb[:, r, 0:512], in0=pouts0[r],
                                 in1=x2_tiles[r][:, 0:512])
        for r in range(2):
            nc.vector.tensor_add(out=o_sb[:, r, 512:1024], in0=pouts1[r],
                                 in1=x2_tiles[r][:, 512:1024])
            if b2_bc is not None:
                nc.vector.tensor_add(out=o_sb[:, r, :], in0=o_sb[:, r, :],
                                     in1=b2_bc)
            nc.sync.dma_start(out=out_r[:, r, :], in_=o_sb[:, r, :])

    # col-half 1, rows 2-3: trail the main loop with their own banks
    with tc.tile_pool(name="pso2", bufs=2, space="PSUM") as PSO2:
        for r in range(2, NRB):
            po1 = PSO2.tile([128, 512], F32, tag="out1", name=f"po1_{r}")
            for h in range(NHID):
                g, hh = h // 4, h % 4
                nc.tensor.matmul(po1,
                                 f_tiles[h][:, r * 128:(r + 1) * 128],
                                 w2g1s[g][:, hh, :],
                                 start=(h == 0), stop=(h == NHID - 1))
            nc.vector.tensor_add(out=o_sb[:, r, 512:1024], in0=po1,
                                 in1=x2_tiles[r][:, 512:1024])
            if b2_bc is not None:
                nc.vector.tensor_add(out=o_sb[:, r, :], in0=o_sb[:, r, :],
                                     in1=b2_bc)
            nc.sync.dma_start(out=out_r[:, r, :], in_=o_sb[:, r, :])


def _build_ffn(general_ln: bool, has_b1: bool, has_b2: bool, alpha: float):
    nc = bacc.Bacc("TRN2", target_bir_lowering=False, debug=False)
    x2in = nc.dram_tensor("x2", [RPC, C], F32, kind="ExternalInput").ap()
    w1 = nc.dram_tensor("w1", [NHID // 4, 128, 4, NCC, 128], BF16,
                        kind="ExternalInput").ap()
    w2 = nc.dram_tensor("w2", [4 * C, C], BF16, kind="ExternalInput").ap()
    b1 = ln2w = ln2b = b2 = None
    if has_b1:
        b1 = nc.dram_tensor("b1", [4 * C], F32, kind="ExternalInput").ap()
    if general_ln:
        ln2w = nc.dram_tensor("ln2w", [C], F32, kind="ExternalInput").ap()
        ln2b = nc.dram_tensor("ln2b", [C], F32, kind="ExternalInput").ap()
    if has_b2:
        b2 = nc.dram_tensor("b2", [C], F32, kind="ExternalInput").ap()
    out = nc.dram_tensor("out", [RPC, C], F32, kind="ExternalOutput").ap()
    with tile.TileContext(nc) as tc:
        with ExitStack() as ctx:
            _ffn_body(ctx, tc, x2in, w1, w2, b1, ln2w, ln2b,
                      b2, alpha, out)
    nc.compile()
    return nc


# --------------------------------------------------------------------------
# host orchestration
# --------------------------------------------------------------------------

_NC_CACHE = {}

# Dev-only: KBENCH_TRACE=1 makes each launch profile itself; per-launch
# (name, exec_time_ns, trace_path) land in BENCH_LOG. Off for grading.
TRACE = bool(os.environ.get("KBENCH_TRACE"))
BENCH_LOG = []


def _run(nc, in_maps, name):
    res = run_bass_kernel_spmd(nc, in_maps, list(range(NCORES)), trace=TRACE)
    if TRACE:
        tp = res.instructions_and_trace[1] if res.instructions_and_trace \
            else None
        BENCH_LOG.append((name, res.exec_time_ns, tp))
    return res


def _get_attn_nc(general_ln):
    key = ("attn", general_ln)
    if key not in _NC_CACHE:
        _NC_CACHE[key] = _build_attn(general_ln)
    return _NC_CACHE[key]


def _get_ffn_nc(general_ln, has_b1, has_b2, alpha):
    key = ("ffn", general_ln, has_b1, has_b2, float(alpha))
    if key not in _NC_CACHE:
        _NC_CACHE[key] = _build_ffn(general_ln, has_b1, has_b2,
                                    float(alpha))
    return _NC_CACHE[key]


def attn_in_maps(x_flat, Wq, Wk, Wv, Wo, trivial, ln1_w, ln1_b):
    in_maps = []
    wq_b = [_f8(np.concatenate([Wq[h] for h in range(hg * 8, hg * 8 + 8)],
                               axis=1), QS) for hg in range(2)]
    wk_b = [_f8(np.concatenate([Wk[h] for h in range(hg * 8, hg * 8 + 8)],
                               axis=1), QS) for hg in range(2)]
    wv_b = [_f8(np.concatenate([Wv[h] for h in range(hg * 8, hg * 8 + 8)],
                               axis=1), QS) for hg in range(2)]
    woh_b = [_bf(Wo[hg * 512:(hg + 1) * 512]) for hg in range(2)]
    for c in range(NCORES):
        b, hg = c // 2, c % 2
        m = {
            "x": np.ascontiguousarray(x_flat[b * T:(b + 1) * T]),
            "wq": wq_b[hg],
            "wk": wk_b[hg],
            "wv": wv_b[hg],
            "woh": woh_b[hg],
        }
        if not trivial:
            m["lnw"] = ln1_w
            m["lnb"] = ln1_b
        in_maps.append(m)
    return in_maps


def run_attn(x_flat, Wq, Wk, Wv, Wo, ln1_w, ln1_b):
    """Returns proj_full [B*T, C] f32: attention output @ Wo, summed
    from the per-core half-partials."""
    trivial = bool(np.all(ln1_w == 1.0) and np.all(ln1_b == 0.0))
    nc = _get_attn_nc(not trivial)
    in_maps = attn_in_maps(x_flat, Wq, Wk, Wv, Wo, trivial, ln1_w, ln1_b)
    res = _run(nc, in_maps, "attn")
    proj_full = np.zeros((B * T, C), dtype=np.float32)
    for c in range(NCORES):
        b = c // 2
        pz = res.results[c]["ppart"]
        proj_full[b * T:(b + 1) * T] += pz[0].astype(np.float32)
        proj_full[b * T:(b + 1) * T] += pz[1].astype(np.float32)
    return proj_full


def _w1_arranged(W1):
    # [C, 4C] -> [g, p, hh, cc, q] (h = 4g+hh) so each 4-chunk group is
    # one contiguous 1MB DMA with 8KB per partition line
    a = _bf(W1).reshape(NCC, 128, NHID, 128).transpose(2, 1, 0, 3)
    return np.ascontiguousarray(
        a.reshape(NHID // 4, 4, 128, NCC, 128).transpose(0, 2, 1, 3, 4))


def ffn_in_maps(x2_full, W1, b1, W2, b2, ln2_w, ln2_b, flags):
    trivial, has_b1, has_b2 = flags
    w1_b = _w1_arranged(W1)
    w2_b = _bf(W2)
    in_maps = []
    for c in range(NCORES):
        sl = slice(RPC * c, RPC * (c + 1))
        m = {
            "x2": np.ascontiguousarray(x2_full[sl]),
            "w1": w1_b,
            "w2": w2_b,
        }
        if has_b1:
            m["b1"] = b1
        if not trivial:
            m["ln2w"] = ln2_w
            m["ln2b"] = ln2_b
        if has_b2:
            m["b2"] = b2
        in_maps.append(m)
    return in_maps


def run_ffn(x_flat, proj_full, bo, W1, b1, W2, b2, ln2_w, ln2_b, alpha):
    x2_full = (x_flat + proj_full + np.asarray(bo, np.float32)
               ).astype(np.float32)
    trivial = bool(np.all(ln2_w == 1.0) and np.all(ln2_b == 0.0))
    has_b1 = bool(np.any(b1 != 0.0))
    has_b2 = bool(np.any(b2 != 0.0))
    nc = _get_ffn_nc(not trivial, has_b1, has_b2, alpha)
    flags = (trivial, has_b1, has_b2)
    in_maps = ffn_in_maps(x2_full, W1, b1, W2, b2, ln2_w, ln2_b, flags)
    res = _run(nc, in_maps, "ffn")
    return np.concatenate(
        [res.results[c]["out"] for c in range(NCORES)], axis=0)


def kernel(x, ln1_w, ln1_b, Wk, Wq, Wv, Wo, bo, ln2_w, ln2_b, W1, b1,
           prelu_a, W2, b2):
    x = np.asarray(x, np.float32)
    x_flat = np.ascontiguousarray(x.reshape(B * T, C))
    Wq = np.asarray(Wq, np.float32)
    Wk = np.asarray(Wk, np.float32)
    Wv = np.asarray(Wv, np.float32)
    Wo = np.asarray(Wo, np.float32)
    alpha = float(np.asarray(prelu_a))

    proj_full = run_attn(x_flat, Wq, Wk, Wv, Wo,
                         np.asarray(ln1_w, np.float32),
                         np.asarray(ln1_b, np.float32))
    out = run_ffn(x_flat, proj_full, np.asarray(bo, np.float32),
                  np.asarray(W1, np.float32), np.asarray(b1, np.float32),
                  np.asarray(W2, np.float32), np.asarray(b2, np.float32),
                  np.asarray(ln2_w, np.float32),
                  np.asarray(ln2_b, np.float32), alpha)
    return out.reshape(B, T, C).astype(np.float32)



# revision 89
# speedup vs baseline: 1.1569x; 1.1569x over previous
"""Trainium2 Bass kernel for a dense pre-LN transformer block.

B=4, T=1024, C=1024, H=16 heads (head_size 64).

Distribution over the 8 NeuronCores (two SPMD launches, host-side
reshuffle between them):

  Launch A (attention + partial proj): core c handles batch c//2 and
  head-group c%2 (8 heads). Each core LNs only its own batch, computes
  its heads' QKV (fp8 DoubleRow), scores (bf16), softmax (exp on ACT,
  fp8 out), AV (fp8), and the partial output projection for its 512
  Wo rows; the half-partials stream to DRAM.
  NOTE the reference computes scores as k @ q^T (roles of q/k swapped
  vs standard attention) — handled by using q rows as score partitions.

  Host: proj_full = sum of the 4 partials per batch; x2 = x + proj + bo.

  Launch B (FFN, row-parallel): core c runs LN2 + W1/PReLU/W2 (bf16;
  fp8 W1/W2 measured over the 2e-2 error gate) + residual on rows
  [512c, 512(c+1)).

LayerNorm applies on the vector engine. PReLU(x) = x + (1-a)*Relu(-x),
split ACT/DVE (one PSUM read each). fp8 weights are pre-scaled by
QS=64 to clear e4m3 subnormals; the scale cancels via the exp scale
(q,k) and the scaled ones-column in the softmax denominator (v).
"""

import os
from contextlib import ExitStack

import numpy as np

import concourse.bass as bass
import concourse.tile as tile
from concourse import bacc, mybir
from concourse.bass_utils import run_bass_kernel_spmd
from concourse.masks import make_identity

F32 = mybir.dt.float32
F32R = mybir.dt.float32r
BF16 = mybir.dt.bfloat16
F8 = mybir.dt.float8e4
DR = mybir.MatmulPerfMode.DoubleRow
AF = mybir.ActivationFunctionType
ALU = mybir.AluOpType

# fp8(e4m3) weight pre-scale: lifts the 0.02-rms weights out of the
# subnormal range (min normal 2^-6). For QKV the q/k factors cancel in
# softmax via the exp scale; the v factor cancels against the scaled
# ones-column in the softmax denominator.
QS = 64.0

B, T, C, H, HS = 4, 1024, 1024, 16, 64
NCORES = 8
EPS = 1e-5
SCALE = float(C) ** -0.5  # 1/32, folded into the softmax exp
NEG = -1e30

NTB = T // 128   # 8 token blocks per batch
NCC = C // 128   # 8 channel chunks
HPC = 8          # heads per core


def _bf(x):
    import ml_dtypes
    return np.ascontiguousarray(np.asarray(x, np.float32).astype(
        ml_dtypes.bfloat16))


def _f8(x, scale=1.0):
    import ml_dtypes
    return np.ascontiguousarray(
        (np.asarray(x, np.float32) * scale).astype(ml_dtypes.float8_e4m3))


# --------------------------------------------------------------------------
# kernel A: attention, one batch + 8 heads per core
# --------------------------------------------------------------------------

def _attn_body(ctx, tc, x, wq, wk, wv, lnw, lnb, woh, ppart):
    """Per-core: LN1 on its batch, QKV/scores/AV for its 8 heads, then
    the partial output projection for its head-group channels.

    Scores are built transposed (s on partitions, t on free dim) so the
    softmax denominator comes from an appended ones-column in v; AV
    output lands as [t, d] tiles which are normalized four heads at a
    time (strided reciprocal + broadcast multiply), then PE-transposed
    into catT (kept in SBUF). Each half's catT rows immediately feed
    partial-proj matmuls against the core's 512 Wo rows; the two
    half-partials stream to DRAM and the host sums the four partials
    per batch (launch B then skips proj entirely).
    """
    nc = tc.nc
    general_ln = lnw is not None

    const = ctx.enter_context(tc.tile_pool(name="const", bufs=1))
    scratch = const.tile([128, 128], F32)
    make_identity(nc, scratch)
    ident = const.tile([128, 128], BF16)
    nc.vector.tensor_copy(out=ident, in_=scratch)
    # transposed causal 0/1 mask for diagonal blocks: keep s<=t
    trilT = const.tile([128, 128], BF16)
    nc.gpsimd.memset(trilT, 1.0)
    nc.gpsimd.affine_select(
        out=trilT, in_=trilT, compare_op=ALU.is_ge, fill=0.0, base=0,
        pattern=[[1, 128]], channel_multiplier=-1)
    trilT8 = const.tile([128, 128], F8)
    nc.vector.tensor_copy(out=trilT8, in_=trilT)
    eps_t = const.tile([128, 1], F32)
    nc.vector.memset(eps_t, EPS)

    wq_sb = const.tile([128, 4, 2, 512], F8, tag="wq")
    wk_sb = const.tile([128, 4, 2, 512], F8, tag="wk")
    wv_sb = const.tile([128, 4, 2, 512], F8, tag="wv")
    # qkv weights ride scalar right behind the first odd x tile: wq
    # lands ~3us in, so the first QKV quarter isn't gated on the 2MB of
    # even x tiles ahead of it on sync
    with tc.tile_wait_until(0.0015):
        nc.scalar.dma_start(
            out=wq_sb, in_=wq.rearrange("(a j p) d -> p a j d", j=2, p=128))
        nc.scalar.dma_start(
            out=wk_sb, in_=wk.rearrange("(a j p) d -> p a j d", j=2, p=128))
        nc.scalar.dma_start(
            out=wv_sb, in_=wv.rearrange("(a j p) d -> p a j d", j=2, p=128))
    woh_sb = const.tile([128, 4, C], BF16, tag="woh")
    with tc.tile_wait_until(0.008):
        nc.scalar.dma_start(
            out=woh_sb, in_=woh.rearrange("(dc p) n -> p dc n", p=128))
    if general_ln:
        lnw_bc = const.tile([128, C], F32, tag="lnw")
        lnb_bc = const.tile([128, C], F32, tag="lnb")
        nc.sync.dma_start(
            out=lnw_bc,
            in_=bass.AP(tensor=lnw.tensor, offset=lnw.offset,
                        ap=[[0, 128]] + list(lnw.ap)))
        nc.sync.dma_start(
            out=lnb_bc,
            in_=bass.AP(tensor=lnb.tensor, offset=lnb.offset,
                        ap=[[0, 128]] + list(lnb.ap)))

    xp = ctx.enter_context(tc.tile_pool(name="xp", bufs=3))
    hp = ctx.enter_context(tc.tile_pool(name="hp", bufs=3))
    hTp = ctx.enter_context(tc.tile_pool(name="hTp", bufs=1))
    stat = ctx.enter_context(tc.tile_pool(name="stat", bufs=3))
    qkp = ctx.enter_context(tc.tile_pool(name="qkp", bufs=3))
    vp = ctx.enter_context(tc.tile_pool(name="vp", bufs=3))
    epl = ctx.enter_context(tc.tile_pool(name="epl", bufs=5))
    ctkp = ctx.enter_context(tc.tile_pool(name="ctkp", bufs=NTB))
    recp = ctx.enter_context(tc.tile_pool(name="recp", bufs=4))
    catp = ctx.enter_context(tc.tile_pool(name="catp", bufs=1))
    ppsb = ctx.enter_context(tc.tile_pool(name="ppsb", bufs=2))

    # 4 PSM banks let QKV keep one accumulation group per bank while the
    # stationary fp8 weight tile is reused across all four token
    # quarters (one LDWEIGHTS per 4 matmuls). Proj shares PAV slots.
    PSM = ctx.enter_context(tc.tile_pool(name="psm", bufs=4, space="PSUM"))
    PSS = ctx.enter_context(tc.tile_pool(name="pss", bufs=2, space="PSUM"))
    PAV = ctx.enter_context(tc.tile_pool(name="pav", bufs=2, space="PSUM"))

    # ---- LN1, fully pipelined per token tile ----
    hT = hTp.tile([128, NCC, T], F8, tag="hT")
    for i in range(NTB):
        xt = xp.tile([128, C], F32, tag="x", name=f"x_{i}")
        # alternate HW DMA queues so early tiles land without queueing
        # behind the full 4MB of x (gpsimd's software queue is slow and
        # stalls the engine that builds ident/trilT)
        (nc.sync if i % 2 == 0 else nc.scalar).dma_start(
            out=xt, in_=x[i * 128:(i + 1) * 128, :])
        st = stat.tile([128, 2, 6], F32, tag="bn", name=f"bn_{i}")
        for k in range(2):
            nc.vector.bn_stats(out=st[:, k, :], in_=xt[:, k * 512:(k + 1) * 512])
        mv = stat.tile([128, 2], F32, tag="mv", name=f"mv_{i}")
        nc.vector.bn_aggr(out=mv, in_=st)
        std = stat.tile([128, 1], F32, tag="std", name=f"std_{i}")
        nc.scalar.activation(out=std, in_=mv[:, 1:2], func=AF.Sqrt, bias=eps_t)
        rstd = stat.tile([128, 1], F32, tag="rstd", name=f"rstd_{i}")
        nc.vector.reciprocal(out=rstd, in_=std)
        ht = hp.tile([128, C], BF16, tag="h", name=f"h_{i}")
        if general_ln:
            hf = hp.tile([128, C], F32, tag="hf", name=f"hf_{i}")
            nc.vector.tensor_scalar(
                out=hf, in0=xt, scalar1=mv[:, 0:1],
                scalar2=rstd, op0=ALU.subtract, op1=ALU.mult)
            nc.vector.tensor_mul(out=hf, in0=hf, in1=lnw_bc)
            nc.vector.tensor_add(out=ht, in0=hf, in1=lnb_bc)
        else:
            nc.vector.tensor_scalar(
                out=ht, in0=xt, scalar1=mv[:, 0:1],
                scalar2=rstd, op0=ALU.subtract, op1=ALU.mult)
        # transpose this tile into hT right away: 8 cc blocks -> one bank
        pt = PSM.tile([128, 1024], BF16, tag="mm", name=f"pt_{i}")
        for cc in range(NCC):
            nc.tensor.transpose(
                pt[:, cc * 128:(cc + 1) * 128],
                ht[:, cc * 128:(cc + 1) * 128], ident)
        nc.vector.tensor_copy(
            out=hT[:, :, i * 128:(i + 1) * 128],
            in_=pt.rearrange("p (cc q) -> p cc q", cc=NCC))

    cat_toks = [ctkp.tile([128, 512], BF16, tag="ctk", name=f"ctk_{i}")
                for i in range(NTB)]
    catT = catp.tile([128, 4, T], BF16, tag="catT")

    # ---- per half (= 2 head pairs): qkv, scores, AV, norm ----
    for half in range(2):
        e_all = {}   # (h4, sc) -> e tile
        v2s = []
        for pp_ in range(2):
            p = half * 2 + pp_
            psl = slice(p * 128, (p + 1) * 128)
            qT2 = qkp.tile([128, T], BF16, tag="qT", name=f"qT_{p}")
            kT2 = qkp.tile([128, T], BF16, tag="kT", name=f"kT_{p}")

            def qkv_mm(w_sb, dst, nm, on_act):
                # fp8 DoubleRow, a4-outer: each stationary weight tile
                # is loaded once and fed all four 256-token quarters
                # (one open accumulation group per PSUM bank - safe)
                pqs = [PSM.tile([128, 256], F32, tag="mm",
                                name=f"p{nm}_{p}_{tq}") for tq in range(4)]
                for a4 in range(4):
                    for tq in range(4):
                        nc.tensor.matmul(
                            pqs[tq], w_sb[:, a4, :, psl],
                            hT[:, 2 * a4:2 * a4 + 2,
                               tq * 256:(tq + 1) * 256],
                            start=(a4 == 0), stop=(a4 == 3), perf_mode=DR)
                for tq in range(4):
                    osl = slice(tq * 256, (tq + 1) * 256)
                    if on_act:
                        nc.scalar.copy(out=dst[:, osl], in_=pqs[tq])
                    else:
                        nc.vector.tensor_copy(out=dst[:, osl], in_=pqs[tq])

            qkv_mm(wq_sb, qT2, "q", True)
            qkv_mm(wk_sb, kT2, "k", True)
            # v2: [t_part, sc, 132]: per head 66 cols (64 v + ones + pad).
            # Compute vT with wide matmuls (stationary wv reused), then
            # PE-transpose blocks back to [t, d] for the AV rhs.
            vT2 = qkp.tile([128, T], BF16, tag="vT", name=f"vT_{p}")
            qkv_mm(wv_sb, vT2, "v", False)
            v2 = vp.tile([128, NTB, 132], F8, tag="v2", name=f"v2_{p}")
            nc.vector.memset(v2[:, :, 64:66], 0.0)
            nc.vector.memset(v2[:, :, 130:132], 0.0)
            # ones-column carries the same QS scale as v so the
            # normalization by the denominator cancels it exactly
            nc.vector.memset(v2[:, :, 64:65], QS)
            nc.vector.memset(v2[:, :, 130:131], QS)
            for g in range(2):
                ptv = PSM.tile([128, 512], BF16, tag="mm", name=f"ptv_{p}_{g}")
                for j in range(4):
                    i = g * 4 + j
                    nc.tensor.transpose(
                        ptv[:, j * 128:(j + 1) * 128],
                        vT2[:, i * 128:(i + 1) * 128], ident)
                pv4 = ptv.rearrange("p (j two d) -> p j two d", j=4, two=2)
                nc.vector.tensor_copy(
                    out=v2[:, g * 4:(g + 1) * 4, 0:64], in_=pv4[:, :, 0, :])
                nc.vector.tensor_copy(
                    out=v2[:, g * 4:(g + 1) * 4, 66:130], in_=pv4[:, :, 1, :])
            v2s.append(v2)

            # scores + exp for the two heads of this pair. e lands fp8 in
            # E[h4] laid out [scpair, j, i, t] so AV can run DoubleRow
            # over s-block pairs.
            for hh in range(2):
                h4 = pp_ * 2 + hh
                hsl = slice(hh * 64, (hh + 1) * 64)
                Eh = epl.tile([128, 4, 2, NTB, 128], F8, tag="E",
                              name=f"E_{half}_{h4}")
                e_all[h4] = Eh
                for sc in range(NTB):
                    W = (NTB - sc) * 128
                    g, j = sc // 2, sc % 2
                    n0 = 0
                    while n0 < W:
                        n1 = min(n0 + 512, W)
                        pss = PSS.tile([128, n1 - n0], F32, tag="score",
                                       name=f"pss_{p}_{hh}_{sc}_{n0}")
                        nc.tensor.matmul(
                            pss,
                            qT2[hsl, sc * 128:(sc + 1) * 128],
                            kT2[hsl, sc * 128 + n0:sc * 128 + n1],
                            start=True, stop=True)
                        nc.scalar.activation(
                            out=Eh[:, g, j, sc + n0 // 128:sc + n1 // 128,
                                   :],
                            in_=pss, func=AF.Exp,
                            scale=SCALE / (QS * QS))
                        n0 = n1
                    # causal mask on the diagonal block: cheap fp8 SBUF
                    # multiply after exp (exp of unmasked scores is benign)
                    nc.vector.tensor_mul(out=Eh[:, g, j, sc, :],
                                         in0=Eh[:, g, j, sc, :], in1=trilT8)

        # AV: 4 heads batched into one [128, 264] psum per token block.
        # Plain fp8 matmuls: DoubleRow here loses — the paired stationary
        # doubles LDWEIGHTS, which dominates these 66-row matmuls.
        for i in range(NTB):
            po4 = PAV.tile([128, 4, 66], F32, tag="po4", name=f"po4_{half}_{i}")
            for h4 in range(4):
                hh = h4 % 2
                v2 = v2s[h4 // 2]
                Eh = e_all[h4]
                for sc in range(i + 1):
                    nc.tensor.matmul(
                        po4[:, h4, :],
                        Eh[:, sc // 2, sc % 2, i, :],
                        v2[:, sc, hh * 66:(hh + 1) * 66],
                        start=(sc == 0), stop=(sc == i))
            rec4 = recp.tile([128, 4], F32, tag="rec", name=f"rec_{half}_{i}")
            nc.vector.reciprocal(out=rec4, in_=po4[:, :, 64:65].rearrange(
                "p a b -> p (a b)"))
            rec_bc = bass.AP(tensor=rec4.tensor, offset=rec4.offset,
                             ap=[list(rec4.ap[0]), [1, 4], [0, 64]])
            nc.vector.tensor_tensor(
                out=cat_toks[i][:, half * 256:(half + 1) * 256].rearrange(
                    "p (a b) -> p a b", a=4),
                in0=po4[:, :, 0:64], in1=rec_bc, op=ALU.mult)

        # transpose this half's cat columns -> catT rows (SBUF only)
        for ccc in (2 * half, 2 * half + 1):
            for g in range(2):
                pt = PSM.tile([128, 512], BF16, tag="mm",
                              name=f"ptc_{ccc}_{g}")
                for j in range(4):
                    i = g * 4 + j
                    nc.tensor.transpose(
                        pt[:, j * 128:(j + 1) * 128],
                        cat_toks[i][:, ccc * 128:(ccc + 1) * 128], ident)
                nc.vector.tensor_copy(
                    out=catT[:, ccc, g * 512:(g + 1) * 512], in_=pt)
        # partial proj for this half's two d-chunks: overlaps the other
        # half's (ACT-bound) score/AV work, and launch B drops proj.
        # All 16 pieces gather in one SBUF tile -> a single 2MB DMA.
        pph = ppsb.tile([128, 2, NTB, 512], BF16, tag="pph",
                        name=f"pph_{half}")
        ppart_r = ppart[half].rearrange("(i p) (co n) -> p co i n",
                                        p=128, co=2)
        for co in range(2):
            for i in range(NTB):
                ppp = PSS.tile([128, 512], F32, tag="score",
                               name=f"pp_{half}_{i}_{co}")
                for k, dc in enumerate((2 * half, 2 * half + 1)):
                    nc.tensor.matmul(
                        ppp, catT[:, dc, i * 128:(i + 1) * 128],
                        woh_sb[:, dc, co * 512:(co + 1) * 512],
                        start=(k == 0), stop=(k == 1))
                # half1's copies land in the kernel tail where exp is
                # done — use the freed ACT engine there, DVE otherwise
                if half == 1:
                    nc.scalar.copy(out=pph[:, co, i, :], in_=ppp)
                else:
                    nc.vector.tensor_copy(out=pph[:, co, i, :], in_=ppp)
                if i == 3:  # first half of the rows streams out early
                    nc.scalar.dma_start(out=ppart_r[:, co, 0:4],
                                        in_=pph[:, co, 0:4])
            nc.scalar.dma_start(out=ppart_r[:, co, 4:8],
                                in_=pph[:, co, 4:8])


def _build_attn(general_ln: bool):
    nc = bacc.Bacc("TRN2", target_bir_lowering=False, debug=False)
    x = nc.dram_tensor("x", [T, C], F32, kind="ExternalInput").ap()
    wq = nc.dram_tensor("wq", [C, 512], F8, kind="ExternalInput").ap()
    wk = nc.dram_tensor("wk", [C, 512], F8, kind="ExternalInput").ap()
    wv = nc.dram_tensor("wv", [C, 512], F8, kind="ExternalInput").ap()
    lnw = lnb = None
    if general_ln:
        lnw = nc.dram_tensor("lnw", [C], F32, kind="ExternalInput").ap()
        lnb = nc.dram_tensor("lnb", [C], F32, kind="ExternalInput").ap()
    woh = nc.dram_tensor("woh", [512, C], BF16, kind="ExternalInput").ap()
    ppart = nc.dram_tensor("ppart", [2, T, C], BF16,
                           kind="ExternalOutput").ap()
    with tile.TileContext(nc) as tc:
        with ExitStack() as ctx:
            _attn_body(ctx, tc, x, wq, wk, wv, lnw, lnb, woh, ppart)
    nc.compile()
    return nc


# --------------------------------------------------------------------------
# kernel B: FFN, 512 rows per core
# --------------------------------------------------------------------------

RPC = (B * T) // NCORES  # 512 rows per core
NRB = RPC // 128         # 4 row blocks
NHID = 4 * C // 128      # 32 hidden chunks


def _ffn_body(ctx, tc, x2in, w1, w2, b1, ln2w, ln2b, b2, alpha, out):
    """Per-core rows: LN2 + FFN on host-precomputed x2 = x + proj + bo.
    The projection itself happens in launch A."""
    nc = tc.nc
    general_ln = ln2w is not None

    const = ctx.enter_context(tc.tile_pool(name="const", bufs=1))
    scratch = const.tile([128, 128], F32)
    make_identity(nc, scratch)
    ident = const.tile([128, 128], BF16)
    nc.vector.tensor_copy(out=ident, in_=scratch)
    eps_t = const.tile([128, 1], F32)
    nc.vector.memset(eps_t, EPS)
    b1_sb = None
    if b1 is not None:
        b1_sb = const.tile([128, NHID], F32, tag="b1")
        nc.sync.dma_start(out=b1_sb, in_=b1.rearrange("(h p) -> p h", p=128))

    def bcast(src, tag):
        t = const.tile([128, C], F32, tag=tag, name=tag)
        nc.sync.dma_start(
            out=t, in_=bass.AP(tensor=src.tensor, offset=src.offset,
                               ap=[[0, 128]] + list(src.ap)))
        return t

    lnw_bc = bcast(ln2w, "lnw") if general_ln else None
    lnb_bc = bcast(ln2b, "lnb") if general_ln else None
    b2_bc = bcast(b2, "b2") if b2 is not None else None

    x2p = ctx.enter_context(tc.tile_pool(name="x2p", bufs=NRB))
    hp = ctx.enter_context(tc.tile_pool(name="hp", bufs=5))
    h2Tp = ctx.enter_context(tc.tile_pool(name="h2Tp", bufs=1))
    stat = ctx.enter_context(tc.tile_pool(name="stat", bufs=2))
    w1p = ctx.enter_context(tc.tile_pool(name="w1p", bufs=2))
    w2p = ctx.enter_context(tc.tile_pool(name="w2p", bufs=2))
    w2bp = ctx.enter_context(tc.tile_pool(name="w2bp", bufs=NHID // 4))
    ftp = ctx.enter_context(tc.tile_pool(name="ftp", bufs=NHID))
    tmp = ctx.enter_context(tc.tile_pool(name="tmp", bufs=3))
    osb = ctx.enter_context(tc.tile_pool(name="osb", bufs=1))

    # x2 tiles alternate the two HW queues so LN2 starts immediately
    x2_tiles = []
    for r in range(NRB):
        x2t = x2p.tile([128, C], F32, tag="x2", name=f"x2_{r}")
        (nc.sync if r % 2 == 0 else nc.scalar).dma_start(
            out=x2t, in_=x2in[r * 128:(r + 1) * 128, :])
        x2_tiles.append(x2t)

    # ---- LN2 + transpose, pipelined per row tile ----
    h2T = h2Tp.tile([128, NCC, RPC], BF16, tag="h2T")
    with tc.tile_pool(name="pst", bufs=2, space="PSUM") as PST:
        for r in range(NRB):
            st = stat.tile([128, 2, 6], F32, tag="bn", name=f"bn_{r}")
            for k in range(2):
                nc.vector.bn_stats(out=st[:, k, :],
                                   in_=x2_tiles[r][:, k * 512:(k + 1) * 512])
            mv = stat.tile([128, 2], F32, tag="mv", name=f"mv_{r}")
            nc.vector.bn_aggr(out=mv, in_=st)
            std = stat.tile([128, 1], F32, tag="std", name=f"std_{r}")
            nc.scalar.activation(out=std, in_=mv[:, 1:2], func=AF.Sqrt,
                                 bias=eps_t)
            rstd = stat.tile([128, 1], F32, tag="rstd", name=f"rstd_{r}")
            nc.vector.reciprocal(out=rstd, in_=std)
            ht = hp.tile([128, C], BF16, tag="h", name=f"h_{r}")
            if general_ln:
                hf = hp.tile([128, C], F32, tag="hf", name=f"hf_{r}")
                nc.vector.tensor_scalar(
                    out=hf, in0=x2_tiles[r], scalar1=mv[:, 0:1],
                    scalar2=rstd, op0=ALU.subtract, op1=ALU.mult)
                nc.vector.tensor_mul(out=hf, in0=hf, in1=lnw_bc)
                nc.vector.tensor_add(out=ht, in0=hf, in1=lnb_bc)
            else:
                nc.vector.tensor_scalar(
                    out=ht, in0=x2_tiles[r], scalar1=mv[:, 0:1],
                    scalar2=rstd, op0=ALU.subtract, op1=ALU.mult)
            pt = PST.tile([128, 1024], BF16, tag="tr", name=f"pt_{r}")
            for cc in range(NCC):
                nc.tensor.transpose(pt[:, cc * 128:(cc + 1) * 128],
                                    ht[:, cc * 128:(cc + 1) * 128], ident)
            nc.vector.tensor_copy(
                out=h2T[:, :, r * 128:(r + 1) * 128],
                in_=pt.rearrange("p (cc q) -> p cc q", cc=NCC))

    # ---- W1 + PReLU + W2 (col-half 0), then W2 col-half 1 ----
    # bf16 throughout: fp8(e4m3) DoubleRow was measured at rel err
    # 1.9e-2 (W1 or W2 alone) to 2.7e-2 (both) vs the 2e-2 gate.
    NG = NHID // 4
    f_tiles = []
    w1gs = []
    with tc.tile_wait_until(0.002):
        for g in range(NG):
            w1g = w1p.tile([128, 4, NCC, 128], BF16, tag="w1", name=f"w1_{g}")
            nc.sync.dma_start(out=w1g, in_=w1[g])
            w1gs.append(w1g)
    w2gs0 = []
    with tc.tile_wait_until(0.010):
        for g in range(NG):
            w2g = w2p.tile([128, 4, 512], BF16, tag="w2a", name=f"w2a_{g}")
            nc.scalar.dma_start(
                out=w2g,
                in_=w2[g * 512:(g + 1) * 512, 0:512].rearrange(
                    "(hh p) n -> p hh n", p=128))
            w2gs0.append(w2g)
    w2g1s = []
    with tc.tile_wait_until(0.018):
        for g in range(NG):
            w2g1 = w2bp.tile([128, 4, 512], BF16, tag="w2b", name=f"w2b_{g}")
            nc.scalar.dma_start(
                out=w2g1,
                in_=w2[g * 512:(g + 1) * 512, 512:1024].rearrange(
                    "(hh p) n -> p hh n", p=128))
            w2g1s.append(w2g1)
    out_r = out.rearrange("(r p) c -> p r c", p=128)
    o_sb = osb.tile([128, NRB, C], F32, tag="o")
    # col-half 1 rows 0-1 accumulate inline with the W1 loop (8 PSUM
    # banks: 2 pf + 4 half0 + 2 half1a); only rows 2-3 trail the loop.
    with tc.tile_pool(name="psf", bufs=2, space="PSUM") as PSF, \
         tc.tile_pool(name="pso", bufs=4, space="PSUM") as PSO, \
         tc.tile_pool(name="pso1", bufs=2, space="PSUM") as PSO1:
        pouts0 = [PSO.tile([128, 512], F32, tag="out0", name=f"po0_{r}")
                  for r in range(NRB)]
        pouts1 = [PSO1.tile([128, 512], F32, tag="out1", name=f"po1_{r}")
                  for r in range(2)]
        for h in range(NHID):
            g, hh = h // 4, h % 4
            pf = PSF.tile([128, RPC], F32, tag="ft", name=f"pf_{h}")
            if h < 4:
                # half-row chunks: the first W1 matmuls only need h2T
                # rows 0-1, so PE starts before LN2 finishes rows 2-3
                for tch in range(2):
                    tsl = slice(tch * 256, (tch + 1) * 256)
                    for cc in range(NCC):
                        nc.tensor.matmul(pf[:, tsl], w1gs[g][:, hh, cc, :],
                                         h2T[:, cc, tsl],
                                         start=(cc == 0),
                                         stop=(cc == NCC - 1))
            else:
                for cc in range(NCC):
                    nc.tensor.matmul(pf, w1gs[g][:, hh, cc, :],
                                     h2T[:, cc, :],
                                     start=(cc == 0), stop=(cc == NCC - 1))
            ft = ftp.tile([128, RPC], BF16, tag="ft", name=f"ft_{h}")
            if b1_sb is not None:
                src = tmp.tile([128, RPC], F32, tag="pb", name=f"pb_{h}")
                nc.vector.tensor_scalar_add(out=src, in0=pf,
                                            scalar1=b1_sb[:, h:h + 1])
            else:
                src = pf
            # PReLU(x) = x + (1-a)*Relu(-x); Relu on ACT (one PSUM read),
            # add on DVE (one PSUM read) — BIR allows max one PSUM input.
            rneg = tmp.tile([128, RPC], BF16, tag="rneg", name=f"rn_{h}")
            nc.scalar.activation(out=rneg, in_=src, func=AF.Relu,
                                 scale=alpha - 1.0)
            nc.vector.tensor_add(out=ft, in0=src, in1=rneg)
            f_tiles.append(ft)
            for r in range(NRB):
                nc.tensor.matmul(pouts0[r], ft[:, r * 128:(r + 1) * 128],
                                 w2gs0[g][:, hh, :],
                                 start=(h == 0), stop=(h == NHID - 1))
            for r in range(2):
                nc.tensor.matmul(pouts1[r], ft[:, r * 128:(r + 1) * 128],
                                 w2g1s[g][:, hh, :],
                                 start=(h == 0), stop=(h == NHID - 1))
        for r in range(NRB):
            nc.vector.tensor_add(out=o_sb[:, r, 0:512], in0=pouts0[r],
                                 in1=x2_tiles[r][:, 0:512])
        for r in range(2):
            nc.vector.tensor_add(out=o_sb[:, r, 512:1024], in0=pouts1[r],
                                 in1=x2_tiles[r][:, 512:1024])
            if b2_bc is not None:
                nc.vector.tensor_add(out=o_sb[:, r, :], in0=o_sb[:, r, :],
                                     in1=b2_bc)
            nc.sync.dma_start(out=out_r[:, r, :], in_=o_sb[:, r, :])

    # col-half 1, rows 2-3: trail the main loop with their own banks
    with tc.tile_pool(name="pso2", bufs=2, space="PSUM") as PSO2:
        for r in range(2, NRB):
            po1 = PSO2.tile([128, 512], F32, tag="out1", name=f"po1_{r}")
            for h in range(NHID):
                g, hh = h // 4, h % 4
                nc.tensor.matmul(po1,
                                 f_tiles[h][:, r * 128:(r + 1) * 128],
                                 w2g1s[g][:, hh, :],
                                 start=(h == 0), stop=(h == NHID - 1))
            nc.vector.tensor_add(out=o_sb[:, r, 512:1024], in0=po1,
                                 in1=x2_tiles[r][:, 512:1024])
            if b2_bc is not None:
                nc.vector.tensor_add(out=o_sb[:, r, :], in0=o_sb[:, r, :],
                                     in1=b2_bc)
            nc.sync.dma_start(out=out_r[:, r, :], in_=o_sb[:, r, :])


def _build_ffn(general_ln: bool, has_b1: bool, has_b2: bool, alpha: float):
    nc = bacc.Bacc("TRN2", target_bir_lowering=False, debug=False)
    x2in = nc.dram_tensor("x2", [RPC, C], F32, kind="ExternalInput").ap()
    w1 = nc.dram_tensor("w1", [NHID // 4, 128, 4, NCC, 128], BF16,
                        kind="ExternalInput").ap()
    w2 = nc.dram_tensor("w2", [4 * C, C], BF16, kind="ExternalInput").ap()
    b1 = ln2w = ln2b = b2 = None
    if has_b1:
        b1 = nc.dram_tensor("b1", [4 * C], F32, kind="ExternalInput").ap()
    if general_ln:
        ln2w = nc.dram_tensor("ln2w", [C], F32, kind="ExternalInput").ap()
        ln2b = nc.dram_tensor("ln2b", [C], F32, kind="ExternalInput").ap()
    if has_b2:
        b2 = nc.dram_tensor("b2", [C], F32, kind="ExternalInput").ap()
    out = nc.dram_tensor("out", [RPC, C], F32, kind="ExternalOutput").ap()
    with tile.TileContext(nc) as tc:
        with ExitStack() as ctx:
            _ffn_body(ctx, tc, x2in, w1, w2, b1, ln2w, ln2b,
                      b2, alpha, out)
    nc.compile()
    return nc


# --------------------------------------------------------------------------
# host orchestration
# --------------------------------------------------------------------------

_NC_CACHE = {}

# Dev-only: KBENCH_TRACE=1 makes each launch profile itself; per-launch
# (name, exec_time_ns, trace_path) land in BENCH_LOG. Off for grading.
TRACE = bool(os.environ.get("KBENCH_TRACE"))
BENCH_LOG = []


def _run(nc, in_maps, name):
    res = run_bass_kernel_spmd(nc, in_maps, list(range(NCORES)), trace=TRACE)
    if TRACE:
        tp = res.instructions_and_trace[1] if res.instructions_and_trace \
            else None
        BENCH_LOG.append((name, res.exec_time_ns, tp))
    return res


def _get_attn_nc(general_ln):
    key = ("attn", general_ln)
    if key not in _NC_CACHE:
        _NC_CACHE[key] = _build_attn(general_ln)
    return _NC_CACHE[key]


def _get_ffn_nc(general_ln, has_b1, has_b2, alpha):
    key = ("ffn", general_ln, has_b1, has_b2, float(alpha))
    if key not in _NC_CACHE:
        _NC_CACHE[key] = _build_ffn(general_ln, has_b1, has_b2,
                                    float(alpha))
    return _NC_CACHE[key]


def attn_in_maps(x_flat, Wq, Wk, Wv, Wo, trivial, ln1_w, ln1_b):
    in_maps = []
    wq_b = [_f8(np.concatenate([Wq[h] for h in range(hg * 8, hg * 8 + 8)],
                               axis=1), QS) for hg in range(2)]
    wk_b = [_f8(np.concatenate([Wk[h] for h in range(hg * 8, hg * 8 + 8)],
                               axis=1), QS) for hg in range(2)]
    wv_b = [_f8(np.concatenate([Wv[h] for h in range(hg * 8, hg * 8 + 8)],
                               axis=1), QS) for hg in range(2)]
    woh_b = [_bf(Wo[hg * 512:(hg + 1) * 512]) for hg in range(2)]
    for c in range(NCORES):
        b, hg = c // 2, c % 2
        m = {
            "x": np.ascontiguousarray(x_flat[b * T:(b + 1) * T]),
            "wq": wq_b[hg],
            "wk": wk_b[hg],
            "wv": wv_b[hg],
            "woh": woh_b[hg],
        }
        if not trivial:
            m["lnw"] = ln1_w
            m["lnb"] = ln1_b
        in_maps.append(m)
    return in_maps


def run_attn(x_flat, Wq, Wk, Wv, Wo, ln1_w, ln1_b):
    """Returns proj_full [B*T, C] f32: attention output @ Wo, summed
    from the per-core half-partials."""
    trivial = bool(np.all(ln1_w == 1.0) and np.all(ln1_b == 0.0))
    nc = _get_attn_nc(not trivial)
    in_maps = attn_in_maps(x_flat, Wq, Wk, Wv, Wo, trivial, ln1_w, ln1_b)
    res = _run(nc, in_maps, "attn")
    proj_full = np.zeros((B * T, C), dtype=np.float32)
    for c in range(NCORES):
        b = c // 2
        pz = res.results[c]["ppart"]
        proj_full[b * T:(b + 1) * T] += pz[0].astype(np.float32)
        proj_full[b * T:(b + 1) * T] += pz[1].astype(np.float32)
    return proj_full


def _w1_arranged(W1):
    # [C, 4C] -> [g, p, hh, cc, q] (h = 4g+hh) so each 4-chunk group is
    # one contiguous 1MB DMA with 8KB per partition line
    a = _bf(W1).reshape(NCC, 128, NHID, 128).transpose(2, 1, 0, 3)
    return np.ascontiguousarray(
        a.reshape(NHID // 4, 4, 128, NCC, 128).transpose(0, 2, 1, 3, 4))


def ffn_in_maps(x2_full, W1, b1, W2, b2, ln2_w, ln2_b, flags):
    trivial, has_b1, has_b2 = flags
    w1_b = _w1_arranged(W1)
    w2_b = _bf(W2)
    in_maps = []
    for c in range(NCORES):
        sl = slice(RPC * c, RPC * (c + 1))
        m = {
            "x2": np.ascontiguousarray(x2_full[sl]),
            "w1": w1_b,
            "w2": w2_b,
        }
        if has_b1:
            m["b1"] = b1
        if not trivial:
            m["ln2w"] = ln2_w
            m["ln2b"] = ln2_b
        if has_b2:
            m["b2"] = b2
        in_maps.append(m)
    return in_maps


def run_ffn(x_flat, proj_full, bo, W1, b1, W2, b2, ln2_w, ln2_b, alpha):
    x2_full = (x_flat + proj_full + np.asarray(bo, np.float32)
               ).astype(np.float32)
    trivial = bool(np.all(ln2_w == 1.0) and np.all(ln2_b == 0.0))
    has_b1 = bool(np.any(b1 != 0.0))
    has_b2 = bool(np.any(b2 != 0.0))
    nc = _get_ffn_nc(not trivial, has_b1, has_b2, alpha)
    flags = (trivial, has_b1, has_b2)
    in_maps = ffn_in_maps(x2_full, W1, b1, W2, b2, ln2_w, ln2_b, flags)
    res = _run(nc, in_maps, "ffn")
    return np.concatenate(
        [res.results[c]["out"] for c in range(NCORES)], axis=0)


def kernel(x, ln1_w, ln1_b, Wk, Wq, Wv, Wo, bo, ln2_w, ln2_b, W1, b1,
           prelu_a, W2, b2):
    x = np.asarray(x, np.float32)
    x_flat = np.ascontiguousarray(x.reshape(B * T, C))
    Wq = np.asarray(Wq, np.float32)
    Wk = np.asarray(Wk, np.float32)
    Wv = np.asarray(Wv, np.float32)
    Wo = np.asarray(Wo, np.float32)
    alpha = float(np.asarray(prelu_a))

    proj_full = run_attn(x_flat, Wq, Wk, Wv, Wo,
                         np.asarray(ln1_w, np.float32),
                         np.asarray(ln1_b, np.float32))
    out = run_ffn(x_flat, proj_full, np.asarray(bo, np.float32),
                  np.asarray(W1, np.float32), np.asarray(b1, np.float32),
                  np.asarray(W2, np.float32), np.asarray(b2, np.float32),
                  np.asarray(ln2_w, np.float32),
                  np.asarray(ln2_b, np.float32), alpha)
    return out.reshape(B, T, C).astype(np.float32)

